# revision 1
# baseline (speedup 1.0000x reference)
"""Causal multi-head attention (B=4, S=2048, H=2048, NH=16) on 8 TRN2 NeuronCores.

Strategy (tensor-parallel over heads + all-to-all reshard):
  - Each core owns 2 heads. Host slices W_attn/b_attn per core, casts to
    bf16 and pre-transposes x (fp32 accumulation happens in PSUM).
  - Phase A (per batch): QKV projection from x^T tiles produces Q^T, K^T
    ([head_dim, tokens] — ready as scores operands) and V ([tokens,
    head_dim] with interleaved ones columns — ready as the PV moving
    operand carrying a free softmax denominator).
  - Phase B (per batch, per head): scores^T = K^T.T @ Q^T on causal
    blocks only; exp on ScalarE straight out of PSUM (no max
    subtraction — scores are bounded); P^T tiles are PV stationary
    operands, rhs = [V | ones] so the denominator accumulates in PSUM
    column 128. Normalize with per-row reciprocal on VectorE.
  - One AllToAll per batch reshards attention output from head-parallel
    to token-parallel; emission order interleaves A/B/C across batches
    so collectives and the ScalarE exp stream overlap PE work.
  - Phase C: exact output projection for this core's 256-token slice of
    each batch (full hidden contraction is local after the A2A); b_proj
    enters via a rank-1 ones matmul.

Self-contained: hardcodes all shapes; no file reads.
"""

import numpy as np
import ml_dtypes

import concourse.bacc as bacc
import concourse.tile as tile
import concourse.mybir as mybir
from concourse import bass_utils

BF16 = mybir.dt.bfloat16
F32 = mybir.dt.float32
AF = mybir.ActivationFunctionType

N_CORES = 8
B = 4
S = 2048
H = 2048
NH = 16
HD = 128
HPC = NH // N_CORES          # heads per core = 2
TOK = B * S                  # 8192
KCH = H // 128               # 16 hidden chunks
SC = 512                     # token chunk for projections / q-chunks
TPB_CH = S // SC             # 4 token chunks per batch
QB = S // 128                # 16 q/kv blocks per batch
SCALE = 1.0 / float(np.sqrt(HD))
VSTRIDE = 2 * (HD + 1)       # V storage: per tokblock [Vh0|1|Vh1|1]
TPB = S // N_CORES           # 256 tokens per core per batch after A2A

_CACHE: dict = {}
LAST_RESULT = None


def _build():
    nc = bacc.Bacc("TRN2", target_bir_lowering=False, debug=False,
                   num_devices=N_CORES)
    xT = nc.dram_tensor("xT", [H, TOK], BF16, kind="ExternalInput")
    wqkv = nc.dram_tensor("wqkv", [H, 6 * HD], BF16, kind="ExternalInput")
    wproj = nc.dram_tensor("wproj", [H, H], BF16, kind="ExternalInput")
    bqkv = nc.dram_tensor("bqkv", [1, 6 * HD], BF16, kind="ExternalInput")
    bqk_t = nc.dram_tensor("bqk_t", [128, 4], F32, kind="ExternalInput")
    bproj = nc.dram_tensor("bproj", [1, H], BF16, kind="ExternalInput")
    mask = nc.dram_tensor("mask", [128, 128], BF16, kind="ExternalInput")
    out = nc.dram_tensor("out", [B * TPB, H], F32, kind="ExternalOutput")

    with tile.TileContext(nc) as tc:
        with (
            tc.tile_pool(name="const", bufs=1) as constp,
            tc.tile_pool(name="qkp", bufs=8) as qkp,
            tc.tile_pool(name="vsp", bufs=2) as vsp,
            tc.tile_pool(name="wpstore", bufs=1) as wpstore,
            tc.tile_pool(name="dram", bufs=1, space="DRAM") as dram,
            tc.tile_pool(name="xTp", bufs=20) as xTp,
            tc.tile_pool(name="wqp", bufs=1) as wqp,
            tc.tile_pool(name="psA", bufs=2, space="PSUM") as psA,
            tc.tile_pool(name="psS", bufs=4, space="PSUM") as psS,
            tc.tile_pool(name="ptP", bufs=20) as ptP,
            tc.tile_pool(name="anP", bufs=4) as anP,
            tc.tile_pool(name="recP", bufs=4) as recP,
            tc.tile_pool(name="atP", bufs=4) as atP,
            tc.tile_pool(name="psC", bufs=2, space="PSUM") as psC,
            tc.tile_pool(name="outP", bufs=3) as outP,
        ):
            mask_sb = constp.tile([128, 128], BF16, name="mask_sb")
            nc.sync.dma_start(mask_sb[:], mask[:])
            ones_sb = constp.tile([1, 128], BF16, name="ones_sb")
            nc.vector.memset(ones_sb[:], 1.0)
            bqkv_sb = constp.tile([1, 6 * HD], BF16, name="bqkv_sb")
            nc.sync.dma_start(bqkv_sb[:], bqkv[:])
            bqkt_sb = constp.tile([128, 4], F32, name="bqkt_sb")
            nc.sync.dma_start(bqkt_sb[:], bqk_t[:])
            bproj_sb = constp.tile([1, H], BF16, name="bproj_sb")
            nc.sync.dma_start(bproj_sb[:], bproj[:])

            # W_qkv, resident (sync queue; first x tiles go on scalar).
            wt = [wqp.tile([128, 6 * HD], BF16, name=f"wt{kc}")
                  for kc in range(KCH)]
            for kc in range(KCH):
                nc.sync.dma_start(wt[kc][:],
                                  wqkv[kc * 128:(kc + 1) * 128, :])
            # W_proj, resident [128, H] per hidden chunk (loaded after
            # phase_a(0) is emitted so it doesn't delay the first x tiles).
            wpt = [wpstore.tile([128, H], BF16, name=f"wpt{kc}")
                   for kc in range(KCH)]

            def load_wproj():
                for kc in range(KCH):
                    nc.scalar.dma_start(wpt[kc][:],
                                        wproj[kc * 128:(kc + 1) * 128, :])

            # Per-batch Q^T/K^T and V stores, recycled through shared-tag
            # pools with 2 batches of depth (B(b) frees slots for A(b+2)).
            qk_store = [None] * B
            vstore = [None] * B

            a2a_in = [dram.tile([S, HPC * HD], BF16, name=f"cc_in{b}")
                      for b in range(B)]
            a2a_out = [dram.tile([S, HPC * HD], BF16, name=f"cc_out{b}")
                       for b in range(B)]

            def phase_a(b):
                """QKV projection for batch b."""
                qk_store[b] = [qkp.tile([128, S], BF16, name="qkt")
                               for _ in range(4)]
                vstore[b] = vsp.tile([128, QB * VSTRIDE], BF16, name="vst")
                nc.vector.memset(vstore[b][:], 1.0)
                for tloc in range(TPB_CH):
                    t = b * TPB_CH + tloc
                    xt = []
                    for kc in range(KCH):
                        xtile = xTp.tile([128, SC], BF16, name="xt")
                        eng = nc.sync if kc % 2 == 0 else nc.scalar
                        eng.dma_start(
                            xtile[:],
                            xT[kc * 128:(kc + 1) * 128, t * SC:(t + 1) * SC])
                        xt.append(xtile)
                    for ob in range(4):      # q_h0, q_h1, k_h0, k_h1
                        ps = psA.tile([128, SC], F32, name="psa")
                        for kc in range(KCH):
                            nc.tensor.matmul(
                                ps[:],
                                wt[kc][:, ob * 128:(ob + 1) * 128],
                                xt[kc][:],
                                start=(kc == 0), stop=(kc == KCH - 1))
                        nc.vector.tensor_scalar_add(
                            qk_store[b][ob][:, tloc * SC:(tloc + 1) * SC],
                            ps[:], bqkt_sb[:, ob:ob + 1])
                    for tb in range(4):      # V blocks, natural layout
                        psw = psA.tile([128, SC], F32, name="psa")
                        ps = psw[:, 0:2 * HD]
                        for kc in range(KCH):
                            nc.tensor.matmul(
                                ps,
                                xt[kc][:, tb * 128:(tb + 1) * 128],
                                wt[kc][:, 4 * HD:6 * HD],
                                start=(kc == 0), stop=False)
                        nc.tensor.matmul(ps, ones_sb[:],
                                         bqkv_sb[:, 4 * HD:6 * HD],
                                         start=False, stop=True)
                        base = (tloc * 4 + tb) * VSTRIDE
                        nc.vector.tensor_copy(
                            vstore[b][:, base:base + HD], ps[:, 0:HD])
                        nc.vector.tensor_copy(
                            vstore[b][:, base + HD + 1:base + 2 * HD + 1],
                            ps[:, HD:2 * HD])

            def phase_b(b):
                """Attention for batch b (both heads) + its AllToAll."""
                for h in range(HPC):
                    qt = qk_store[b][h]
                    kt = qk_store[b][2 + h]
                    for qc in range(4):
                        pts = []
                        for kb in range(4 * (qc + 1)):
                            col0 = max(0, kb * 128 - qc * SC)
                            ps = psS.tile([128, SC], F32, name="pss")
                            nc.tensor.matmul(
                                ps[:, col0:SC],
                                kt[:, kb * 128:(kb + 1) * 128],
                                qt[:, qc * SC + col0:(qc + 1) * SC],
                                start=True, stop=True)
                            pt = ptP.tile([128, SC], BF16, name="pt")
                            nc.scalar.activation(
                                pt[:, col0:SC], ps[:, col0:SC],
                                AF.Exp, scale=SCALE)
                            if kb >= 4 * qc:
                                nc.vector.tensor_mul(
                                    pt[:, col0:col0 + 128],
                                    pt[:, col0:col0 + 128],
                                    mask_sb[:])
                            pts.append(pt)
                        for qb in range(4):
                            qg = qc * 4 + qb
                            po = psS.tile([128, SC], F32, name="pss")[:, 0:HD + 1]
                            for kb in range(qg + 1):
                                vbase = kb * VSTRIDE + h * (HD + 1)
                                nc.tensor.matmul(
                                    po[:],
                                    pts[kb][:, qb * 128:(qb + 1) * 128],
                                    vstore[b][:, vbase:vbase + HD + 1],
                                    start=(kb == 0), stop=(kb == qg))
                            rec = recP.tile([128, 1], F32, name="rec")
                            nc.vector.reciprocal(rec[:], po[:, HD:HD + 1])
                            an = anP.tile([128, HD], BF16, name="an")
                            nc.vector.tensor_scalar_mul(
                                an[:], po[:, 0:HD], rec[:])
                            nc.sync.dma_start(
                                a2a_in[b][qg * 128:(qg + 1) * 128,
                                          h * HD:(h + 1) * HD],
                                an[:])
                nc.gpsimd.collective_compute(
                    "AllToAll",
                    mybir.AluOpType.bypass,
                    replica_groups=[list(range(N_CORES))],
                    ins=[a2a_in[b].opt()],
                    outs=[a2a_out[b].opt()],
                )

            def phase_c(b):
                """Output projection for this core's token slice of batch b."""
                # Two wide DMA-transposes bring the whole received buffer in
                # as a^T: partition p = hidden-within-head-half, free =
                # shard*TPB + token. lhsT for chunk hc lives at free offset
                # (hc//2)*TPB in at_w[hc%2].
                at_w = []
                for half in range(2):
                    atile = atP.tile([128, S], BF16, name="at")
                    nc.sync.dma_start(
                        atile[:],
                        a2a_out[b][:, half * 128:(half + 1) * 128],
                        transpose=True)
                    at_w.append(atile)
                for oc in range(4):
                    for tb in range(TPB // 128):
                        ps = psC.tile([128, SC], F32, name="psc")
                        for hc in range(KCH):
                            nc.tensor.matmul(
                                ps[:],
                                at_w[hc % 2][:, (hc // 2) * TPB
                                             + tb * 128:
                                             (hc // 2) * TPB + (tb + 1) * 128],
                                wpt[hc][:, oc * SC:(oc + 1) * SC],
                                start=(hc == 0), stop=False)
                        nc.tensor.matmul(
                            ps[:], ones_sb[:],
                            bproj_sb[:, oc * SC:(oc + 1) * SC],
                            start=False, stop=True)
                        ot = outP.tile([128, SC], F32, name="ot")
                        nc.vector.tensor_copy(ot[:], ps[:])
                        nc.scalar.dma_start(
                            out[b * TPB + tb * 128:b * TPB + (tb + 1) * 128,
                                oc * SC:(oc + 1) * SC],
                            ot[:])

            # Interleaved emission: overlap A/B/C across batches so the
            # in-order PE stream never waits on an A2A, and ScalarE's exp
            # work spreads across the whole kernel.
            phase_a(0)
            load_wproj()
            phase_a(1)
            phase_b(0)
            phase_a(2)
            phase_b(1)
            phase_c(0)
            phase_a(3)
            phase_b(2)
            phase_c(1)
            phase_c(2)
            phase_b(3)
            phase_c(3)

    nc.compile()
    return nc


def _get_nc():
    if "nc" not in _CACHE:
        _CACHE["nc"] = _build()
    return _CACHE["nc"]


def kernel(hidden_states, W_attn, b_attn, W_proj, b_proj):
    global LAST_RESULT
    bf = ml_dtypes.bfloat16
    x = np.asarray(hidden_states, dtype=np.float32).reshape(TOK, H)
    # bf16 cast then a fast uint16 transpose copy -> x^T [H, TOK]
    xb = x.astype(bf)
    xT = np.ascontiguousarray(xb.view(np.uint16).T).view(bf)
    Wa = np.asarray(W_attn, dtype=np.float32)
    ba = np.asarray(b_attn, dtype=np.float32)
    Wp = np.ascontiguousarray(np.asarray(W_proj, dtype=np.float32)).astype(bf)
    bp = np.asarray(b_proj, dtype=np.float32).reshape(1, H).astype(bf)
    mask = np.triu(np.ones((128, 128), dtype=np.float32)).astype(bf)

    in_maps = []
    for c in range(N_CORES):
        h0 = c * HPC
        cols = []
        for part in range(3):          # q, k, v feature slices
            cols.append(np.arange(part * H + h0 * HD,
                                  part * H + (h0 + HPC) * HD))
        cols = np.concatenate(cols)    # 768 column indices
        wq = np.ascontiguousarray(Wa[:, cols]).astype(bf)
        bq = ba[cols].reshape(1, 6 * HD).astype(bf)
        # per-partition bias for the 4 Q^T/K^T feature blocks
        bqk_t = np.ascontiguousarray(
            ba[cols[:4 * 128]].reshape(4, 128).T).astype(np.float32)
        in_maps.append({
            "xT": xT,
            "wqkv": wq,
            "wproj": Wp,
            "bqkv": bq,
            "bqk_t": bqk_t,
            "bproj": bp,
            "mask": mask,
        })

    nc = _get_nc()
    res = bass_utils.run_bass_kernel_spmd(
        nc, in_maps, core_ids=list(range(N_CORES)))
    LAST_RESULT = res

    full = np.empty((B, S, H), dtype=np.float32)
    for c in range(N_CORES):
        r = res.results[c]["out"]
        for b in range(B):
            full[b, c * TPB:(c + 1) * TPB, :] = r[b * TPB:(b + 1) * TPB, :]
    return full



# revision 5
# speedup vs baseline: 1.0600x; 1.0600x over previous
"""Causal multi-head attention (B=4, S=2048, H=2048, NH=16) on 8 TRN2 NeuronCores.

Strategy (tensor-parallel over heads + all-to-all reshard):
  - Each core owns 2 heads. Host slices W_attn/b_attn per core, casts to
    bf16 and pre-transposes x (fp32 accumulation happens in PSUM).
  - Phase A (per batch): QKV projection from x^T tiles produces Q^T, K^T
    ([head_dim, tokens] — ready as scores operands) and V ([tokens,
    head_dim] with interleaved ones columns — ready as the PV moving
    operand carrying a free softmax denominator).
  - Phase B (per batch, per head): scores^T = K^T.T @ Q^T on causal
    blocks only; kv-blocks are PAIRED into [128,1024] PSUM tiles (2
    banks) so each ScalarE exp covers two blocks (halves the ACT fixed
    cost — ScalarE is the B-phase co-bottleneck). P^T tiles are PV
    stationary operands, rhs = [V | ones] so the denominator
    accumulates in PSUM column 128. Normalize with per-row reciprocal
    on VectorE.
  - One AllToAll per batch reshards attention output from head-parallel
    to token-parallel. Emission order A0 A1 B0 A2 B1 A3 B2 C0 B3 C1 C2
    C3 keeps >40us of independent PE work after every A2A trigger
    (including batch 3's), so the in-order PE stream never waits on a
    collective.
  - Phase C: exact output projection for this core's 256-token slice of
    each batch; b_proj enters via a rank-1 ones matmul.
  - Queues: sync = x-loads + even an-stores + C transposes/stores;
    scalar = exp + wproj + odd an-stores; gpsimd = collective triggers.

Self-contained: hardcodes all shapes; no file reads.
"""

import numpy as np
import ml_dtypes

import concourse.bacc as bacc
import concourse.tile as tile
import concourse.mybir as mybir
from concourse import bass_utils

BF16 = mybir.dt.bfloat16
F32 = mybir.dt.float32
AF = mybir.ActivationFunctionType

N_CORES = 8
B = 4
S = 2048
H = 2048
NH = 16
HD = 128
HPC = NH // N_CORES          # heads per core = 2
TOK = B * S                  # 8192
KCH = H // 128               # 16 hidden chunks
SC = 512                     # token chunk for projections / q-chunks
TPB_CH = S // SC             # 4 token chunks per batch
QB = S // 128                # 16 q/kv blocks per batch
SCALE = 1.0 / float(np.sqrt(HD))
VSTRIDE = 2 * (HD + 1)       # V storage: per tokblock [Vh0|1|Vh1|1]
TPB = S // N_CORES           # 256 tokens per core per batch after A2A

_CACHE: dict = {}
LAST_RESULT = None


def _build():
    nc = bacc.Bacc("TRN2", target_bir_lowering=False, debug=False,
                   num_devices=N_CORES)
    xT = nc.dram_tensor("xT", [H, TOK], BF16, kind="ExternalInput")
    wqkv = nc.dram_tensor("wqkv", [H, 6 * HD], BF16, kind="ExternalInput")
    wproj = nc.dram_tensor("wproj", [H, H], BF16, kind="ExternalInput")
    bqkv = nc.dram_tensor("bqkv", [1, 6 * HD], BF16, kind="ExternalInput")
    bqk_t = nc.dram_tensor("bqk_t", [128, 4], F32, kind="ExternalInput")
    bproj = nc.dram_tensor("bproj", [1, H], BF16, kind="ExternalInput")
    mask = nc.dram_tensor("mask", [128, 128], BF16, kind="ExternalInput")
    out = nc.dram_tensor("out", [B * TPB, H], F32, kind="ExternalOutput")

    with tile.TileContext(nc) as tc:
        with (
            tc.tile_pool(name="const", bufs=1) as constp,
            tc.tile_pool(name="qkp", bufs=8) as qkp,
            tc.tile_pool(name="vsp", bufs=2) as vsp,
            tc.tile_pool(name="wpstore", bufs=1) as wpstore,
            tc.tile_pool(name="dram", bufs=1, space="DRAM") as dram,
            tc.tile_pool(name="xTp", bufs=24) as xTp,
            tc.tile_pool(name="wqp", bufs=1) as wqp,
            tc.tile_pool(name="psA", bufs=2, space="PSUM") as psA,
            tc.tile_pool(name="psS", bufs=2, space="PSUM") as psS,
            tc.tile_pool(name="ptP", bufs=12) as ptP,
            tc.tile_pool(name="anP", bufs=4) as anP,
            tc.tile_pool(name="recP", bufs=4) as recP,
            tc.tile_pool(name="atP", bufs=3) as atP,
            tc.tile_pool(name="psC", bufs=2, space="PSUM") as psC,
            tc.tile_pool(name="outP", bufs=2) as outP,
        ):
            mask_sb = constp.tile([128, 128], BF16, name="mask_sb")
            nc.sync.dma_start(mask_sb[:], mask[:])
            ones_sb = constp.tile([1, 128], BF16, name="ones_sb")
            nc.vector.memset(ones_sb[:], 1.0)
            bqkv_sb = constp.tile([1, 6 * HD], BF16, name="bqkv_sb")
            nc.sync.dma_start(bqkv_sb[:], bqkv[:])
            bqkt_sb = constp.tile([128, 4], F32, name="bqkt_sb")
            nc.sync.dma_start(bqkt_sb[:], bqk_t[:])
            bproj_sb = constp.tile([1, H], BF16, name="bproj_sb")
            nc.sync.dma_start(bproj_sb[:], bproj[:])

            # W_qkv resident; tiles interleave with the first x chunk so
            # the first matmul's operands arrive ASAP.
            wt = [wqp.tile([128, 6 * HD], BF16, name=f"wt{kc}")
                  for kc in range(KCH)]
            # W_proj, resident [128, H] per hidden chunk (emitted after
            # phase_a(0) on the scalar queue — loads during A0 compute).
            wpt = [wpstore.tile([128, H], BF16, name=f"wpt{kc}")
                   for kc in range(KCH)]

            def load_wproj():
                for kc in range(KCH):
                    nc.scalar.dma_start(wpt[kc][:],
                                        wproj[kc * 128:(kc + 1) * 128, :])

            # Per-batch Q^T/K^T and V stores, recycled through shared-tag
            # pools with 2 batches of depth.
            qk_store = [None] * B
            vstore = [None] * B

            a2a_in = [dram.tile([S, HPC * HD], BF16, name=f"cc_in{b}")
                      for b in range(B)]
            a2a_out = [dram.tile([S, HPC * HD], BF16, name=f"cc_out{b}")
                       for b in range(B)]

            def phase_a(b):
                """QKV projection for batch b."""
                qk_store[b] = [qkp.tile([128, S], BF16, name="qkt")
                               for _ in range(4)]
                vstore[b] = vsp.tile([128, QB * VSTRIDE], BF16, name="vst")
                nc.vector.memset(vstore[b][:], 1.0)
                for tloc in range(TPB_CH):
                    t = b * TPB_CH + tloc
                    xt = []
                    for kc in range(KCH):
                        xtile = xTp.tile([128, SC], BF16, name="xt")
                        nc.sync.dma_start(
                            xtile[:],
                            xT[kc * 128:(kc + 1) * 128, t * SC:(t + 1) * SC])
                        if b == 0 and tloc == 0:
                            # interleave the resident W_qkv loads with the
                            # first x chunk (same queue, fair share)
                            nc.sync.dma_start(
                                wt[kc][:], wqkv[kc * 128:(kc + 1) * 128, :])
                        xt.append(xtile)
                    for ob in range(4):      # q_h0, q_h1, k_h0, k_h1
                        ps = psA.tile([128, SC], F32, name="psa")
                        for kc in range(KCH):
                            nc.tensor.matmul(
                                ps[:],
                                wt[kc][:, ob * 128:(ob + 1) * 128],
                                xt[kc][:],
                                start=(kc == 0), stop=(kc == KCH - 1))
                        nc.vector.tensor_scalar_add(
                            qk_store[b][ob][:, tloc * SC:(tloc + 1) * SC],
                            ps[:], bqkt_sb[:, ob:ob + 1])
                    for tb in range(4):      # V blocks, natural layout
                        psw = psA.tile([128, SC], F32, name="psa")
                        ps = psw[:, 0:2 * HD]
                        for kc in range(KCH):
                            nc.tensor.matmul(
                                ps,
                                xt[kc][:, tb * 128:(tb + 1) * 128],
                                wt[kc][:, 4 * HD:6 * HD],
                                start=(kc == 0), stop=False)
                        nc.tensor.matmul(ps, ones_sb[:],
                                         bqkv_sb[:, 4 * HD:6 * HD],
                                         start=False, stop=True)
                        base = (tloc * 4 + tb) * VSTRIDE
                        nc.vector.tensor_copy(
                            vstore[b][:, base:base + HD], ps[:, 0:HD])
                        nc.vector.tensor_copy(
                            vstore[b][:, base + HD + 1:base + 2 * HD + 1],
                            ps[:, HD:2 * HD])

            def phase_b(b):
                """Attention for batch b (both heads) + its AllToAll.

                kv-blocks are processed in pairs sharing one [128,1024]
                PSUM tile (2 banks) and one exp ACTIVATE.
                """
                for h in range(HPC):
                    qt = qk_store[b][h]
                    kt = qk_store[b][2 + h]
                    for qc in range(4):
                        npairs = 2 * (qc + 1)
                        pts = []
                        for pr in range(npairs):
                            kb0 = 2 * pr
                            ps = psS.tile([128, 2 * SC], F32, name="pss")
                            pt = ptP.tile([128, 2 * SC], BF16, name="pt")
                            c0s = [max(0, (kb0 + half) * 128 - qc * SC)
                                   for half in range(2)]
                            for half in range(2):
                                kb = kb0 + half
                                off = half * SC
                                nc.tensor.matmul(
                                    ps[:, off + c0s[half]:off + SC],
                                    kt[:, kb * 128:(kb + 1) * 128],
                                    qt[:, qc * SC + c0s[half]:(qc + 1) * SC],
                                    start=True, stop=True)
                            if pr < 2 * qc:
                                # full (non-diagonal) pair: one wide exp
                                nc.scalar.activation(
                                    pt[:], ps[:], AF.Exp, scale=SCALE)
                            else:
                                # diagonal pair: per-half exp + mask
                                for half in range(2):
                                    off = half * SC
                                    c0 = c0s[half]
                                    nc.scalar.activation(
                                        pt[:, off + c0:off + SC],
                                        ps[:, off + c0:off + SC],
                                        AF.Exp, scale=SCALE)
                                    nc.vector.tensor_mul(
                                        pt[:, off + c0:off + c0 + 128],
                                        pt[:, off + c0:off + c0 + 128],
                                        mask_sb[:])
                            pts.append(pt)
                        for qb in range(4):
                            qg = qc * 4 + qb
                            po = psA.tile([128, SC], F32,
                                          name="psa")[:, 0:HD + 1]
                            for kb in range(qg + 1):
                                vbase = kb * VSTRIDE + h * (HD + 1)
                                src = pts[kb // 2][:, (kb % 2) * SC
                                                   + qb * 128:
                                                   (kb % 2) * SC
                                                   + (qb + 1) * 128]
                                nc.tensor.matmul(
                                    po[:],
                                    src,
                                    vstore[b][:, vbase:vbase + HD + 1],
                                    start=(kb == 0), stop=(kb == qg))
                            rec = recP.tile([128, 1], F32, name="rec")
                            nc.vector.reciprocal(rec[:], po[:, HD:HD + 1])
                            an = anP.tile([128, HD], BF16, name="an")
                            nc.vector.tensor_scalar_mul(
                                an[:], po[:, 0:HD], rec[:])
                            eng = nc.sync if qg % 2 == 0 else nc.scalar
                            eng.dma_start(
                                a2a_in[b][qg * 128:(qg + 1) * 128,
                                          h * HD:(h + 1) * HD],
                                an[:])
                nc.gpsimd.collective_compute(
                    "AllToAll",
                    mybir.AluOpType.bypass,
                    replica_groups=[list(range(N_CORES))],
                    ins=[a2a_in[b].opt()],
                    outs=[a2a_out[b].opt()],
                )

            def phase_c(b):
                """Output projection for this core's token slice of batch b."""
                at_w = []
                for half in range(2):
                    atile = atP.tile([128, S], BF16, name="at")
                    nc.sync.dma_start(
                        atile[:],
                        a2a_out[b][:, half * 128:(half + 1) * 128],
                        transpose=True)
                    at_w.append(atile)
                for oc in range(4):
                    for tb in range(TPB // 128):
                        ps = psC.tile([128, SC], F32, name="psc")
                        for hc in range(KCH):
                            nc.tensor.matmul(
                                ps[:],
                                at_w[hc % 2][:, (hc // 2) * TPB
                                             + tb * 128:
                                             (hc // 2) * TPB + (tb + 1) * 128],
                                wpt[hc][:, oc * SC:(oc + 1) * SC],
                                start=(hc == 0), stop=False)
                        nc.tensor.matmul(
                            ps[:], ones_sb[:],
                            bproj_sb[:, oc * SC:(oc + 1) * SC],
                            start=False, stop=True)
                        ot = outP.tile([128, SC], F32, name="ot")
                        nc.vector.tensor_copy(ot[:], ps[:])
                        nc.sync.dma_start(
                            out[b * TPB + tb * 128:b * TPB + (tb + 1) * 128,
                                oc * SC:(oc + 1) * SC],
                            ot[:])

            # Emission order: every A2A trigger is followed by >40us of
            # independent PE work before its consumer phase.
            phase_a(0)
            load_wproj()
            phase_a(1)
            phase_b(0)
            phase_a(2)
            phase_b(1)
            phase_a(3)
            phase_b(2)
            phase_c(0)
            phase_b(3)
            phase_c(1)
            phase_c(2)
            phase_c(3)

    nc.compile()
    return nc


def _get_nc():
    if "nc" not in _CACHE:
        _CACHE["nc"] = _build()
    return _CACHE["nc"]


def kernel(hidden_states, W_attn, b_attn, W_proj, b_proj):
    global LAST_RESULT
    bf = ml_dtypes.bfloat16
    x = np.asarray(hidden_states, dtype=np.float32).reshape(TOK, H)
    # bf16 cast then a fast uint16 transpose copy -> x^T [H, TOK]
    xb = x.astype(bf)
    xT = np.ascontiguousarray(xb.view(np.uint16).T).view(bf)
    Wa = np.asarray(W_attn, dtype=np.float32)
    ba = np.asarray(b_attn, dtype=np.float32)
    Wp = np.ascontiguousarray(np.asarray(W_proj, dtype=np.float32)).astype(bf)
    bp = np.asarray(b_proj, dtype=np.float32).reshape(1, H).astype(bf)
    mask = np.triu(np.ones((128, 128), dtype=np.float32)).astype(bf)

    in_maps = []
    for c in range(N_CORES):
        h0 = c * HPC
        cols = []
        for part in range(3):          # q, k, v feature slices
            cols.append(np.arange(part * H + h0 * HD,
                                  part * H + (h0 + HPC) * HD))
        cols = np.concatenate(cols)    # 768 column indices
        wq = np.ascontiguousarray(Wa[:, cols]).astype(bf)
        bq = ba[cols].reshape(1, 6 * HD).astype(bf)
        # per-partition bias for the 4 Q^T/K^T feature blocks
        bqk_t = np.ascontiguousarray(
            ba[cols[:4 * 128]].reshape(4, 128).T).astype(np.float32)
        in_maps.append({
            "xT": xT,
            "wqkv": wq,
            "wproj": Wp,
            "bqkv": bq,
            "bqk_t": bqk_t,
            "bproj": bp,
            "mask": mask,
        })

    nc = _get_nc()
    res = bass_utils.run_bass_kernel_spmd(
        nc, in_maps, core_ids=list(range(N_CORES)))
    LAST_RESULT = res

    full = np.empty((B, S, H), dtype=np.float32)
    for c in range(N_CORES):
        r = res.results[c]["out"]
        for b in range(B):
            full[b, c * TPB:(c + 1) * TPB, :] = r[b * TPB:(b + 1) * TPB, :]
    return full
